# revision 12
# baseline (speedup 1.0000x reference)
"""Deformable Conv2D (DCNv2-style) on 8 Trainium2 NeuronCores.

Strategy (data-parallel over batch, one sample per core):
  conv-first reformulation:  out[f,j] = sum_kk sum_corner w_corner[kk,j] * Y_kk[f, p_corner(kk,j)]
  where Y_kk = W[:,:,kk] @ x  (plain matmul over all spatial positions).

  Sampling uses a per-tap DRAM table TC[kk] whose row t packs the 4 bilinear
  corner pixel-vectors [Y(t-65) | Y(t-64) | Y(t-1) | Y(t)] (bf16, 1 KB), so a
  single dma_gather descriptor per (tap, output position) fetches all four
  corners.  Gather dispatch on GPSIMD (~7.5 ns/descriptor, 36864 descriptors,
  one ucode core-pair) is the hard critical path; everything else must hide
  under it and must not contend its SBUF port:
    - stage A is ordered kk0-first (then kk{1,2}, {3,4,5}, {6,7,8}) so the
      first gather's table is ready ~40us earlier;
    - the corner combine runs as WIDE DVE ops (one multiply per (kk, corner)
      over all 4096 output positions via a stride-0 broadcast weight view,
      plus wide adds into a bf16 SBUF accumulator) -- narrow per-block ops
      and PE-side accumulation were measured to slow the gather dispatch;
    - psum eviction runs entirely on ACT (the DVE queue is strict
      in-order, so DVE evictions would stall the combine behind them).

  The accumulator keeps j on partitions ([j%128, j//128, f] layout); the
  final [f, j] transpose is done on the host from the raw DMA-out, which
  removes the on-device transpose stage entirely.

Shapes (hardcoded per spec): x (8,128,64,64) f32, offset (8,18,64,64),
mask (8,9,64,64), weight (128,128,3,3), out (8,128,64,64) f32.
"""

import numpy as np
import ml_dtypes
from contextlib import ExitStack

import concourse.bass as bass
import concourse.bacc as bacc
import concourse.tile as tile
from concourse import mybir
from concourse import library_config
from concourse.bass_utils import run_bass_kernel_spmd

B, C, H, W = 8, 128, 64, 64
F = 128
KH = KW = 3
KK = KH * KW
HW = H * W  # 4096
NP = 128
NJB = HW // NP  # 32 j-blocks
NTT = 33  # table row tiles (t in [0, 4224)); gather uses rows [0, 4160]
TROWS = NTT * NP
TCOLS = 4 * F  # 512
# column-slot source shifts: TC[t] = [Y(t-65) | Y(t-64) | Y(t-1) | Y(t)]
SLOT_SHIFT = (-65, -64, -1, 0)
XPAD_LO = 65  # x padding so shifted tiles never index out of range
XPAD = XPAD_LO + TROWS + 64  # padded x columns
# stage-A kk grouping: kk0 alone first so gather kk0 can start early
KK_GROUPS = ((0,), (1, 2), (3, 4, 5), (6, 7, 8))

BF16 = mybir.dt.bfloat16
F32 = mybir.dt.float32
I16 = mybir.dt.int16


def _prep_indices_weights(offset, mask):
    """Per-sample host prep. offset [18,H,W], mask [9,H,W] ->
    idx int16 [128, KK*256], wts bf16 [128, KK*4*NJB]."""
    off = offset.reshape(KK, 2, H, W)
    dy, dx = off[:, 0], off[:, 1]
    ki, kj = np.meshgrid(np.arange(KH), np.arange(KW), indexing="ij")
    ki = ki.reshape(KK, 1, 1).astype(np.float32)
    kj = kj.reshape(KK, 1, 1).astype(np.float32)
    base_y = (np.arange(H, dtype=np.float32) - 1.0)[None, :, None] + ki
    base_x = (np.arange(W, dtype=np.float32) - 1.0)[None, None, :] + kj
    py = base_y + dy
    px = base_x + dx
    y0 = np.floor(py)
    x0 = np.floor(px)
    ly = (py - y0).astype(np.float32)
    lx = (px - x0).astype(np.float32)
    hy = 1.0 - ly
    hx = 1.0 - lx
    y0i = y0.astype(np.int64)
    x0i = x0.astype(np.int64)

    vy0 = (y0i >= 0) & (y0i < H)
    vy1 = (y0i + 1 >= 0) & (y0i + 1 < H)
    vx0 = (x0i >= 0) & (x0i < W)
    vx1 = (x0i + 1 >= 0) & (x0i + 1 < W)

    m = mask.reshape(KK, H, W)
    w00 = (hy * hx * m * (vy0 & vx0)).reshape(KK, HW).astype(np.float32)
    w01 = (hy * lx * m * (vy0 & vx1)).reshape(KK, HW).astype(np.float32)
    w10 = (ly * hx * m * (vy1 & vx0)).reshape(KK, HW).astype(np.float32)
    w11 = (ly * lx * m * (vy1 & vx1)).reshape(KK, HW).astype(np.float32)

    flat = np.clip(y0i * W + x0i + 65, 0, HW + 64).reshape(KK, HW)

    # idx: per kk, 4096 ordinals j wrapped o -> [o%16, o//16], replicated to
    # 128 partitions (dma_gather consumes idxs from each 16-partition group).
    idx_dev = np.empty((128, KK * 256), np.int16)
    for kk in range(KK):
        wrapped = flat[kk].astype(np.int16).reshape(256, 16).T  # [16, 256]
        idx_dev[:, kk * 256 : (kk + 1) * 256] = np.tile(wrapped, (8, 1))

    # wts: [128, (kk, corner, i)]; value[p] = w_c[kk, i*128+p]
    wts_dev = np.empty((128, KK * 4 * NJB), ml_dtypes.bfloat16)
    corners = (w00, w01, w10, w11)
    col = 0
    for kk in range(KK):
        for ci in range(4):
            wc = corners[ci][kk].reshape(NJB, 128)
            wts_dev[:, col : col + NJB] = wc.T
            col += NJB
    return idx_dev, wts_dev


def _split_overfull_waits(nc):
    """This walrus build accepts 1 sync-wait per instruction (2 for EVSEM).
    Move extras onto preceding same-engine NoOps."""
    for f in nc.m.functions:
        for bb in f.blocks:
            new_list = []
            for ins in bb.instructions:
                si = ins.sync_info
                waits = list(si.on_wait) if si and si.on_wait else []
                cap = 2 if isinstance(ins, mybir.InstEventSemaphore) else 1
                if len(waits) > cap:
                    extra, keep = waits[:-cap], waits[-cap:]
                    for k, w in enumerate(extra):
                        nop = mybir.InstNoOp(
                            name=f"{ins.name}_waitsplit{k}",
                            sync_info=mybir.SyncInfo(on_wait=[w], on_update=[]),
                            bass_nofuse=True,
                            engine=ins.engine,
                        )
                        new_list.append(nop)
                        nc.register_instruction(nop, overwrite=True)
                    si.on_wait = keep
                new_list.append(ins)
            bb.instructions[:] = new_list


def _build_nc():
    nc = bacc.Bacc(None, target_bir_lowering=False, debug=False)
    x_d = nc.dram_tensor("x", [NP, XPAD], BF16, kind="ExternalInput")
    wt_d = nc.dram_tensor("wt", [NP, KK * F], BF16, kind="ExternalInput")
    idx_d = nc.dram_tensor("idx", [NP, KK * 256], I16, kind="ExternalInput")
    wts_d = nc.dram_tensor("wts", [NP, KK * 4 * NJB], BF16, kind="ExternalInput")
    out_d = nc.dram_tensor("out", [NP, HW], BF16, kind="ExternalOutput")
    tbl_d = nc.dram_tensor("tbl", [KK, TROWS, TCOLS], BF16, kind="Internal")

    TBL_KK = TROWS * TCOLS

    with tile.TileContext(nc) as tc, ExitStack() as ctx:
        cpool = ctx.enter_context(tc.tile_pool(name="const", bufs=1))
        tcst_pool = ctx.enter_context(tc.tile_pool(name="tcst", bufs=8))
        gpool = ctx.enter_context(tc.tile_pool(name="gat", bufs=3))
        ppool = ctx.enter_context(tc.tile_pool(name="prod", bufs=2))
        accpool = ctx.enter_context(tc.tile_pool(name="acc", bufs=1))
        pspool = ctx.enter_context(tc.tile_pool(name="ps", bufs=2, space="PSUM"))

        x_sb = cpool.tile([NP, XPAD], BF16)
        wt_sb = cpool.tile([NP, KK * F], BF16)
        idx_sb = cpool.tile([NP, KK * 256], I16)
        wts_sb = cpool.tile([NP, KK * 4 * NJB], BF16)
        acc_sb = accpool.tile([NP, NJB, F], BF16)

        # Load the gather ucode library up front so the first dma_gather
        # doesn't pay the ~6us IRAM load mid-pipeline.
        nc.gpsimd.load_library(library_config.mlp)

        nc.sync.dma_start(idx_sb[:], idx_d[:])
        nc.sync.dma_start(x_sb[:], x_d[:])
        nc.sync.dma_start(wt_sb[:], wt_d[:])
        nc.sync.dma_start(wts_sb[:], wts_d[:])

        # ---- Stage A: build TC tables, kk0 first.
        # per (group, tt, slot): stationary = shifted x tile; one matmul over
        # the group's kk's (N = 128*len(group)) -> psum -> evict (cast bf16,
        # 3:1 ACT:DVE) into tcst staging; one DMA per (group, tt) writes the
        # group's table rows.
        for grp in KK_GROUPS:
            ng = len(grp)
            k0 = grp[0]
            for tt in range(NTT):
                tcst = tcst_pool.tile([NP, 3, 4, F], BF16, tag="tcst")
                ps = pspool.tile([NP, 4, 512], F32, tag="ps")
                for s in range(4):
                    xoff = XPAD_LO + tt * NP + SLOT_SHIFT[s]
                    nc.tensor.matmul(
                        ps[:, s, 0 : ng * F],
                        x_sb[:, xoff : xoff + NP],
                        wt_sb[:, k0 * F : (k0 + ng) * F],
                        start=True,
                        stop=True,
                    )
                # evict psum [p, s, (kk, f)] -> tcst [p, kk, s, f].
                # On ACT (the DVE queue is strict in-order, so DVE evictions
                # here would stall the stage-B combine emitted after them) --
                # except kk0, whose evicts finish long before the first
                # combine reaches the DVE queue head: split them ACT/DVE to
                # shorten the critical path to the first gather.
                if ng == 1:
                    s01 = ps[:, 0:2, 0:F].rearrange("p s (k f) -> p s k f", k=1)
                    s23 = ps[:, 2:4, 0:F].rearrange("p s (k f) -> p s k f", k=1)
                    nc.scalar.copy(
                        tcst[:, 0:1, 0:2, :].rearrange("p k s f -> p s k f"), s01
                    )
                    nc.vector.tensor_copy(
                        tcst[:, 0:1, 2:4, :].rearrange("p k s f -> p s k f"), s23
                    )
                else:
                    srcall = ps[:, :, 0 : ng * F].rearrange("p s (k f) -> p s k f", k=ng)
                    nc.scalar.copy(
                        tcst[:, 0:ng, :, :].rearrange("p k s f -> p s k f"), srcall
                    )
                dma_dst = bass.AP(
                    tbl_d,
                    k0 * TBL_KK + tt * NP * TCOLS,
                    [[TCOLS, NP], [TBL_KK, ng], [1, TCOLS]],
                )
                nc.sync.dma_start(dma_dst, tcst[:, 0:ng, :, :])

        # ---- Stage B: gather (2048-idx halves) + wide weighted accumulate
        # on DVE per half -- halves the gather->combine latency and the tail
        # after the last gather, at identical total dispatch cost.
        NH = NJB // 2  # blocks per half
        for kk in range(KK):
            wbase = kk * 4 * NJB
            g_t = gpool.tile([NP, NJB, TCOLS], BF16, tag="g_t")
            src = bass.AP(tbl_d, kk * TBL_KK, [[TCOLS, HW + 65], [1, TCOLS]])
            for hh in range(2):
                nc.gpsimd.dma_gather(
                    out_ap=g_t[:, hh * NH : (hh + 1) * NH, :],
                    in_ap=src,
                    idxs_ap=idx_sb[:, kk * 256 + hh * 128 : kk * 256 + (hh + 1) * 128],
                    num_idxs=HW // 2,
                    num_idxs_reg=HW // 2,
                    elem_size=TCOLS,
                    single_packet=False,
                )
                # acc[p, i, f] += sum_c w_c[p, i] * g_t[p, i, c*F:(c+1)*F]
                for ci in range(4):
                    w_b = (
                        wts_sb[:, wbase + ci * NJB + hh * NH : wbase + ci * NJB + (hh + 1) * NH]
                        .unsqueeze(-1)
                        .broadcast_to((NP, NH, F))
                    )
                    g_c = g_t[:, hh * NH : (hh + 1) * NH, ci * F : (ci + 1) * F]
                    a_h = acc_sb[:, hh * NH : (hh + 1) * NH, :]
                    if kk == 0 and ci == 0:
                        nc.vector.tensor_tensor(a_h, g_c, w_b, mybir.AluOpType.mult)
                    else:
                        prod = ppool.tile([NP, NH, F], BF16, tag="prod")
                        nc.vector.tensor_tensor(prod[:], g_c, w_b, mybir.AluOpType.mult)
                        nc.vector.tensor_tensor(a_h, a_h, prod[:], mybir.AluOpType.add)

        nc.sync.dma_start(out_d[:], acc_sb[:])

    nc.compile()
    _split_overfull_waits(nc)
    return nc


_NC_CACHE = {}


def _get_nc():
    if "nc" not in _NC_CACHE:
        _NC_CACHE["nc"] = _build_nc()
    return _NC_CACHE["nc"]


def _prep_x(xb):
    """x [C,H,W] f32 -> padded bf16 [128, XPAD]."""
    xp = np.zeros((C, XPAD), ml_dtypes.bfloat16)
    xp[:, XPAD_LO : XPAD_LO + HW] = xb.reshape(C, HW).astype(ml_dtypes.bfloat16)
    return xp


def kernel(x, offset, mask, weight, **run_kwargs):
    x = np.asarray(x, np.float32)
    offset = np.asarray(offset, np.float32)
    mask = np.asarray(mask, np.float32)
    weight = np.asarray(weight, np.float32)

    wt = np.transpose(weight.reshape(F, C, KK), (1, 2, 0)).reshape(C, KK * F)
    wt = np.ascontiguousarray(wt).astype(ml_dtypes.bfloat16)

    in_maps = []
    for b in range(B):
        idx_dev, wts_dev = _prep_indices_weights(offset[b], mask[b])
        in_maps.append(
            {
                "x": _prep_x(x[b]),
                "wt": wt,
                "idx": idx_dev,
                "wts": wts_dev,
            }
        )

    nc = _get_nc()
    res = run_bass_kernel_spmd(nc, in_maps, core_ids=list(range(8)), **run_kwargs)
    # out_d[p, i*F + f] holds out[f, j = i*128 + p]: transpose on host.
    outs = []
    for b in range(B):
        arr = np.asarray(res.results[b]["out"]).astype(np.float32)
        arr = arr.reshape(NP, NJB, F)  # [p, i, f]
        outs.append(np.transpose(arr, (2, 1, 0)).reshape(F, H, W))
    out = np.stack(outs)
    if run_kwargs:
        kernel.last_results = res
    return out


# revision 13
# speedup vs baseline: 1.0255x; 1.0255x over previous
"""Deformable Conv2D (DCNv2-style) on 8 Trainium2 NeuronCores.

Strategy (data-parallel over batch, one sample per core):
  conv-first reformulation:  out[f,j] = sum_kk sum_corner w_corner[kk,j] * Y_kk[f, p_corner(kk,j)]
  where Y_kk = W[:,:,kk] @ x  (plain matmul over all spatial positions).

  Sampling uses a per-tap DRAM table TC[kk] whose row t packs the 4 bilinear
  corner pixel-vectors [Y(t-65) | Y(t-64) | Y(t-1) | Y(t)] (bf16, 1 KB), so a
  single dma_gather descriptor per (tap, output position) fetches all four
  corners.  Gather dispatch on GPSIMD (~7.5 ns/descriptor, 36864 descriptors,
  one ucode core-pair) is the hard critical path; everything else must hide
  under it and must not contend its SBUF port:
    - stage A is ordered kk0-first (then kk{1,2}, {3,4,5}, {6,7,8}) so the
      first gather's table is ready ~40us earlier;
    - the corner combine runs as WIDE DVE ops (one multiply per (kk, corner)
      over all 4096 output positions via a stride-0 broadcast weight view,
      plus wide adds into a bf16 SBUF accumulator) -- narrow per-block ops
      and PE-side accumulation were measured to slow the gather dispatch;
    - psum eviction runs entirely on ACT (the DVE queue is strict
      in-order, so DVE evictions would stall the combine behind them).

  The accumulator keeps j on partitions ([j%128, j//128, f] layout); the
  final [f, j] transpose is done on the host from the raw DMA-out, which
  removes the on-device transpose stage entirely.

Shapes (hardcoded per spec): x (8,128,64,64) f32, offset (8,18,64,64),
mask (8,9,64,64), weight (128,128,3,3), out (8,128,64,64) f32.
"""

import numpy as np
import ml_dtypes
from contextlib import ExitStack

import concourse.bass as bass
import concourse.bacc as bacc
import concourse.tile as tile
from concourse import mybir
from concourse import library_config
from concourse.bass_utils import run_bass_kernel_spmd

B, C, H, W = 8, 128, 64, 64
F = 128
KH = KW = 3
KK = KH * KW
HW = H * W  # 4096
NP = 128
NJB = HW // NP  # 32 j-blocks
NTT = 33  # table row tiles (t in [0, 4224)); gather uses rows [0, 4160]
TROWS = NTT * NP
TCOLS = 4 * F  # 512
# column-slot source shifts: TC[t] = [Y(t-65) | Y(t-64) | Y(t-1) | Y(t)]
SLOT_SHIFT = (-65, -64, -1, 0)
XPAD_LO = 65  # x padding so shifted tiles never index out of range
XPAD = XPAD_LO + TROWS + 64  # padded x columns
# stage-A kk grouping: kk0 alone first so gather kk0 can start early
KK_GROUPS = ((0,), (1, 2), (3, 4, 5), (6, 7, 8))

BF16 = mybir.dt.bfloat16
F32 = mybir.dt.float32
I16 = mybir.dt.int16


def _prep_indices_weights(offset, mask):
    """Per-sample host prep. offset [18,H,W], mask [9,H,W] ->
    idx int16 [128, KK*256], wts bf16 [128, KK*4*NJB]."""
    off = offset.reshape(KK, 2, H, W)
    dy, dx = off[:, 0], off[:, 1]
    ki, kj = np.meshgrid(np.arange(KH), np.arange(KW), indexing="ij")
    ki = ki.reshape(KK, 1, 1).astype(np.float32)
    kj = kj.reshape(KK, 1, 1).astype(np.float32)
    base_y = (np.arange(H, dtype=np.float32) - 1.0)[None, :, None] + ki
    base_x = (np.arange(W, dtype=np.float32) - 1.0)[None, None, :] + kj
    py = base_y + dy
    px = base_x + dx
    y0 = np.floor(py)
    x0 = np.floor(px)
    ly = (py - y0).astype(np.float32)
    lx = (px - x0).astype(np.float32)
    hy = 1.0 - ly
    hx = 1.0 - lx
    y0i = y0.astype(np.int64)
    x0i = x0.astype(np.int64)

    vy0 = (y0i >= 0) & (y0i < H)
    vy1 = (y0i + 1 >= 0) & (y0i + 1 < H)
    vx0 = (x0i >= 0) & (x0i < W)
    vx1 = (x0i + 1 >= 0) & (x0i + 1 < W)

    m = mask.reshape(KK, H, W)
    w00 = (hy * hx * m * (vy0 & vx0)).reshape(KK, HW).astype(np.float32)
    w01 = (hy * lx * m * (vy0 & vx1)).reshape(KK, HW).astype(np.float32)
    w10 = (ly * hx * m * (vy1 & vx0)).reshape(KK, HW).astype(np.float32)
    w11 = (ly * lx * m * (vy1 & vx1)).reshape(KK, HW).astype(np.float32)

    flat = np.clip(y0i * W + x0i + 65, 0, HW + 64).reshape(KK, HW)

    # idx: per kk, 4096 ordinals j wrapped o -> [o%16, o//16], replicated to
    # 128 partitions (dma_gather consumes idxs from each 16-partition group).
    idx_dev = np.empty((128, KK * 256), np.int16)
    for kk in range(KK):
        wrapped = flat[kk].astype(np.int16).reshape(256, 16).T  # [16, 256]
        idx_dev[:, kk * 256 : (kk + 1) * 256] = np.tile(wrapped, (8, 1))

    # wts: [128, (kk, corner, i)]; value[p] = w_c[kk, i*128+p]
    wts_dev = np.empty((128, KK * 4 * NJB), ml_dtypes.bfloat16)
    corners = (w00, w01, w10, w11)
    col = 0
    for kk in range(KK):
        for ci in range(4):
            wc = corners[ci][kk].reshape(NJB, 128)
            wts_dev[:, col : col + NJB] = wc.T
            col += NJB
    return idx_dev, wts_dev


def _split_overfull_waits(nc):
    """This walrus build accepts 1 sync-wait per instruction (2 for EVSEM).
    Move extras onto preceding same-engine NoOps."""
    for f in nc.m.functions:
        for bb in f.blocks:
            new_list = []
            for ins in bb.instructions:
                si = ins.sync_info
                waits = list(si.on_wait) if si and si.on_wait else []
                cap = 2 if isinstance(ins, mybir.InstEventSemaphore) else 1
                if len(waits) > cap:
                    extra, keep = waits[:-cap], waits[-cap:]
                    for k, w in enumerate(extra):
                        nop = mybir.InstNoOp(
                            name=f"{ins.name}_waitsplit{k}",
                            sync_info=mybir.SyncInfo(on_wait=[w], on_update=[]),
                            bass_nofuse=True,
                            engine=ins.engine,
                        )
                        new_list.append(nop)
                        nc.register_instruction(nop, overwrite=True)
                    si.on_wait = keep
                new_list.append(ins)
            bb.instructions[:] = new_list


def _build_nc():
    nc = bacc.Bacc(None, target_bir_lowering=False, debug=False)
    x_d = nc.dram_tensor("x", [NP, XPAD], BF16, kind="ExternalInput")
    wt_d = nc.dram_tensor("wt", [NP, KK * F], BF16, kind="ExternalInput")
    idx_d = nc.dram_tensor("idx", [NP, KK * 256], I16, kind="ExternalInput")
    wts_d = nc.dram_tensor("wts", [NP, KK * 4 * NJB], BF16, kind="ExternalInput")
    out_d = nc.dram_tensor("out", [NP, HW], BF16, kind="ExternalOutput")
    tbl_d = nc.dram_tensor("tbl", [KK, TROWS, TCOLS], BF16, kind="Internal")

    TBL_KK = TROWS * TCOLS

    with tile.TileContext(nc) as tc, ExitStack() as ctx:
        cpool = ctx.enter_context(tc.tile_pool(name="const", bufs=1))
        tcst_pool = ctx.enter_context(tc.tile_pool(name="tcst", bufs=8))
        gpool = ctx.enter_context(tc.tile_pool(name="gat", bufs=3))
        ppool = ctx.enter_context(tc.tile_pool(name="prod", bufs=2))
        accpool = ctx.enter_context(tc.tile_pool(name="acc", bufs=1))
        pspool = ctx.enter_context(tc.tile_pool(name="ps", bufs=2, space="PSUM"))

        x_sb = cpool.tile([NP, XPAD], BF16)
        wt_sb = cpool.tile([NP, KK * F], BF16)
        idx_sb = cpool.tile([NP, KK * 256], I16)
        wts_sb = cpool.tile([NP, KK * 4 * NJB], BF16)
        acc_sb = accpool.tile([NP, NJB, F], BF16)

        # Load the gather ucode library up front so the first dma_gather
        # doesn't pay the ~6us IRAM load mid-pipeline.
        nc.gpsimd.load_library(library_config.mlp)

        nc.sync.dma_start(idx_sb[:], idx_d[:])
        nc.sync.dma_start(x_sb[:], x_d[:])
        nc.sync.dma_start(wt_sb[:], wt_d[:])
        nc.sync.dma_start(wts_sb[:], wts_d[:])

        # PE warmup: ~4.5us of dummy matmuls during the input DMAs flips the
        # HAM clock gate to 2.4 GHz before the kk0 table build starts.
        warm = cpool.tile([NP, NP], BF16)
        nc.vector.memset(warm[:], 0)
        wps = pspool.tile([NP, 4, 512], F32, tag="ps")
        for _ in range(42):
            nc.tensor.matmul(wps[:, 0, 0:NP], warm[:], warm[:], start=True, stop=True)

        # ---- Stage A: build TC tables, kk0 first.
        # per (group, tt, slot): stationary = shifted x tile; one matmul over
        # the group's kk's (N = 128*len(group)) -> psum -> evict (cast bf16,
        # 3:1 ACT:DVE) into tcst staging; one DMA per (group, tt) writes the
        # group's table rows.
        for grp in KK_GROUPS:
            ng = len(grp)
            k0 = grp[0]
            for tt in range(NTT):
                tcst = tcst_pool.tile([NP, 3, 4, F], BF16, tag="tcst")
                ps = pspool.tile([NP, 4, 512], F32, tag="ps")
                for s in range(4):
                    xoff = XPAD_LO + tt * NP + SLOT_SHIFT[s]
                    nc.tensor.matmul(
                        ps[:, s, 0 : ng * F],
                        x_sb[:, xoff : xoff + NP],
                        wt_sb[:, k0 * F : (k0 + ng) * F],
                        start=True,
                        stop=True,
                    )
                # evict psum [p, s, (kk, f)] -> tcst [p, kk, s, f].
                # ALL on ACT: the DVE queue is strict in-order, so any DVE
                # eviction here would stall the stage-B combine behind it.
                srcall = ps[:, :, 0 : ng * F].rearrange("p s (k f) -> p s k f", k=ng)
                nc.scalar.copy(
                    tcst[:, 0:ng, :, :].rearrange("p k s f -> p s k f"), srcall
                )
                dma_dst = bass.AP(
                    tbl_d,
                    k0 * TBL_KK + tt * NP * TCOLS,
                    [[TCOLS, NP], [TBL_KK, ng], [1, TCOLS]],
                )
                nc.sync.dma_start(dma_dst, tcst[:, 0:ng, :, :])

        # ---- Stage B: one gather per kk + wide weighted accumulate on DVE.
        # The last kk is gathered in 2048-idx halves so its combine overlaps
        # the second half's dispatch, shortening the post-gather tail.
        NH = NJB // 2  # blocks per half
        for kk in range(KK):
            wbase = kk * 4 * NJB
            g_t = gpool.tile([NP, NJB, TCOLS], BF16, tag="g_t")
            src = bass.AP(tbl_d, kk * TBL_KK, [[TCOLS, HW + 65], [1, TCOLS]])
            halves = 2 if kk == KK - 1 else 1
            nh = NJB // halves
            for hh in range(halves):
                nc.gpsimd.dma_gather(
                    out_ap=g_t[:, hh * nh : (hh + 1) * nh, :],
                    in_ap=src,
                    idxs_ap=idx_sb[
                        :, kk * 256 + hh * (256 // halves) : kk * 256 + (hh + 1) * (256 // halves)
                    ],
                    num_idxs=HW // halves,
                    num_idxs_reg=HW // halves,
                    elem_size=TCOLS,
                    single_packet=False,
                )
                # acc[p, i, f] += sum_c w_c[p, i] * g_t[p, i, c*F:(c+1)*F]
                for ci in range(4):
                    w_b = (
                        wts_sb[:, wbase + ci * NJB + hh * nh : wbase + ci * NJB + (hh + 1) * nh]
                        .unsqueeze(-1)
                        .broadcast_to((NP, nh, F))
                    )
                    g_c = g_t[:, hh * nh : (hh + 1) * nh, ci * F : (ci + 1) * F]
                    a_h = acc_sb[:, hh * nh : (hh + 1) * nh, :]
                    if kk == 0 and ci == 0:
                        nc.vector.tensor_tensor(a_h, g_c, w_b, mybir.AluOpType.mult)
                    else:
                        prod = ppool.tile([NP, nh, F], BF16, tag="prod")
                        nc.vector.tensor_tensor(prod[:], g_c, w_b, mybir.AluOpType.mult)
                        nc.vector.tensor_tensor(a_h, a_h, prod[:], mybir.AluOpType.add)

        nc.sync.dma_start(out_d[:], acc_sb[:])

    nc.compile()
    _split_overfull_waits(nc)
    return nc


_NC_CACHE = {}


def _get_nc():
    if "nc" not in _NC_CACHE:
        _NC_CACHE["nc"] = _build_nc()
    return _NC_CACHE["nc"]


def _prep_x(xb):
    """x [C,H,W] f32 -> padded bf16 [128, XPAD]."""
    xp = np.zeros((C, XPAD), ml_dtypes.bfloat16)
    xp[:, XPAD_LO : XPAD_LO + HW] = xb.reshape(C, HW).astype(ml_dtypes.bfloat16)
    return xp


def kernel(x, offset, mask, weight, **run_kwargs):
    x = np.asarray(x, np.float32)
    offset = np.asarray(offset, np.float32)
    mask = np.asarray(mask, np.float32)
    weight = np.asarray(weight, np.float32)

    wt = np.transpose(weight.reshape(F, C, KK), (1, 2, 0)).reshape(C, KK * F)
    wt = np.ascontiguousarray(wt).astype(ml_dtypes.bfloat16)

    in_maps = []
    for b in range(B):
        idx_dev, wts_dev = _prep_indices_weights(offset[b], mask[b])
        in_maps.append(
            {
                "x": _prep_x(x[b]),
                "wt": wt,
                "idx": idx_dev,
                "wts": wts_dev,
            }
        )

    nc = _get_nc()
    res = run_bass_kernel_spmd(nc, in_maps, core_ids=list(range(8)), **run_kwargs)
    # out_d[p, i*F + f] holds out[f, j = i*128 + p]: transpose on host.
    outs = []
    for b in range(B):
        arr = np.asarray(res.results[b]["out"]).astype(np.float32)
        arr = arr.reshape(NP, NJB, F)  # [p, i, f]
        outs.append(np.transpose(arr, (2, 1, 0)).reshape(F, H, W))
    out = np.stack(outs)
    if run_kwargs:
        kernel.last_results = res
    return out


# revision 15
# speedup vs baseline: 1.0465x; 1.0205x over previous
"""Deformable Conv2D (DCNv2-style) on 8 Trainium2 NeuronCores.

Strategy (data-parallel over batch, one sample per core):
  conv-first reformulation:  out[f,j] = sum_kk sum_corner w_corner[kk,j] * Y_kk[f, p_corner(kk,j)]
  where Y_kk = W[:,:,kk] @ x  (plain matmul over all spatial positions).

  Sampling uses a per-tap DRAM table TC[kk] whose row t packs the 4 bilinear
  corner pixel-vectors [Y(t-65) | Y(t-64) | Y(t-1) | Y(t)] (bf16, 1 KB), so a
  single dma_gather descriptor per (tap, output position) fetches all four
  corners.  Gather dispatch on GPSIMD (~7.5 ns/descriptor, 36864 descriptors,
  one ucode core-pair) is the hard critical path; everything else must hide
  under it and must not contend its SBUF port:
    - stage A is ordered kk0-first (then kk{1,2}, {3,4,5}, {6,7,8}) so the
      first gather's table is ready ~40us earlier;
    - the corner combine runs as WIDE DVE ops (one multiply per (kk, corner)
      over all 4096 output positions via a stride-0 broadcast weight view,
      plus wide adds into a bf16 SBUF accumulator) -- narrow per-block ops
      and PE-side accumulation were measured to slow the gather dispatch;
    - psum eviction runs entirely on ACT (the DVE queue is strict
      in-order, so DVE evictions would stall the combine behind them).

  The accumulator keeps j on partitions ([j%128, j//128, f] layout); the
  final [f, j] transpose is done on the host from the raw DMA-out, which
  removes the on-device transpose stage entirely.

Shapes (hardcoded per spec): x (8,128,64,64) f32, offset (8,18,64,64),
mask (8,9,64,64), weight (128,128,3,3), out (8,128,64,64) f32.
"""

import numpy as np
import ml_dtypes
from contextlib import ExitStack

import concourse.bass as bass
import concourse.bacc as bacc
import concourse.tile as tile
from concourse import mybir
from concourse import library_config
from concourse.bass_utils import run_bass_kernel_spmd

B, C, H, W = 8, 128, 64, 64
F = 128
KH = KW = 3
KK = KH * KW
HW = H * W  # 4096
NP = 128
NJB = HW // NP  # 32 j-blocks
NTT = 33  # table row tiles (t in [0, 4224)); gather uses rows [0, 4160]
TROWS = NTT * NP
TCOLS = 4 * F  # 512
# column-slot source shifts: TC[t] = [Y(t-65) | Y(t-64) | Y(t-1) | Y(t)]
SLOT_SHIFT = (-65, -64, -1, 0)
XPAD_LO = 65  # x padding so shifted tiles never index out of range
XPAD = XPAD_LO + TROWS + 64  # padded x columns
# stage-A kk grouping: kk0 alone first so gather kk0 can start early
KK_GROUPS = ((0,), (1, 2), (3, 4, 5), (6, 7, 8))  # [0] built separately

BF16 = mybir.dt.bfloat16
F32 = mybir.dt.float32
I16 = mybir.dt.int16


def _prep_indices_weights(offset, mask):
    """Per-sample host prep. offset [18,H,W], mask [9,H,W] ->
    idx int16 [128, KK*256], wts bf16 [128, KK*4*NJB]."""
    off = offset.reshape(KK, 2, H, W)
    dy, dx = off[:, 0], off[:, 1]
    ki, kj = np.meshgrid(np.arange(KH), np.arange(KW), indexing="ij")
    ki = ki.reshape(KK, 1, 1).astype(np.float32)
    kj = kj.reshape(KK, 1, 1).astype(np.float32)
    base_y = (np.arange(H, dtype=np.float32) - 1.0)[None, :, None] + ki
    base_x = (np.arange(W, dtype=np.float32) - 1.0)[None, None, :] + kj
    py = base_y + dy
    px = base_x + dx
    y0 = np.floor(py)
    x0 = np.floor(px)
    ly = (py - y0).astype(np.float32)
    lx = (px - x0).astype(np.float32)
    hy = 1.0 - ly
    hx = 1.0 - lx
    y0i = y0.astype(np.int64)
    x0i = x0.astype(np.int64)

    vy0 = (y0i >= 0) & (y0i < H)
    vy1 = (y0i + 1 >= 0) & (y0i + 1 < H)
    vx0 = (x0i >= 0) & (x0i < W)
    vx1 = (x0i + 1 >= 0) & (x0i + 1 < W)

    m = mask.reshape(KK, H, W)
    w00 = (hy * hx * m * (vy0 & vx0)).reshape(KK, HW).astype(np.float32)
    w01 = (hy * lx * m * (vy0 & vx1)).reshape(KK, HW).astype(np.float32)
    w10 = (ly * hx * m * (vy1 & vx0)).reshape(KK, HW).astype(np.float32)
    w11 = (ly * lx * m * (vy1 & vx1)).reshape(KK, HW).astype(np.float32)

    flat = np.clip(y0i * W + x0i + 65, 0, HW + 64).reshape(KK, HW)

    # idx: per kk, 4096 ordinals j wrapped o -> [o%16, o//16], replicated to
    # 128 partitions (dma_gather consumes idxs from each 16-partition group).
    idx_dev = np.empty((128, KK * 256), np.int16)
    for kk in range(KK):
        wrapped = flat[kk].astype(np.int16).reshape(256, 16).T  # [16, 256]
        idx_dev[:, kk * 256 : (kk + 1) * 256] = np.tile(wrapped, (8, 1))

    # wts: [128, (kk, corner, i)]; value[p] = w_c[kk, i*128+p]
    wts_dev = np.empty((128, KK * 4 * NJB), ml_dtypes.bfloat16)
    corners = (w00, w01, w10, w11)
    col = 0
    for kk in range(KK):
        for ci in range(4):
            wc = corners[ci][kk].reshape(NJB, 128)
            wts_dev[:, col : col + NJB] = wc.T
            col += NJB
    return idx_dev, wts_dev


def _split_overfull_waits(nc):
    """This walrus build accepts 1 sync-wait per instruction (2 for EVSEM).
    Move extras onto preceding same-engine NoOps."""
    for f in nc.m.functions:
        for bb in f.blocks:
            new_list = []
            for ins in bb.instructions:
                si = ins.sync_info
                waits = list(si.on_wait) if si and si.on_wait else []
                cap = 2 if isinstance(ins, mybir.InstEventSemaphore) else 1
                if len(waits) > cap:
                    extra, keep = waits[:-cap], waits[-cap:]
                    for k, w in enumerate(extra):
                        nop = mybir.InstNoOp(
                            name=f"{ins.name}_waitsplit{k}",
                            sync_info=mybir.SyncInfo(on_wait=[w], on_update=[]),
                            bass_nofuse=True,
                            engine=ins.engine,
                        )
                        new_list.append(nop)
                        nc.register_instruction(nop, overwrite=True)
                    si.on_wait = keep
                new_list.append(ins)
            bb.instructions[:] = new_list


def _build_nc():
    nc = bacc.Bacc(None, target_bir_lowering=False, debug=False, num_swdge_queues=2)
    x_d = nc.dram_tensor("x", [NP, XPAD], BF16, kind="ExternalInput")
    wt_d = nc.dram_tensor("wt", [NP, KK * F], BF16, kind="ExternalInput")
    idx_d = nc.dram_tensor("idx", [NP, KK * 256], I16, kind="ExternalInput")
    wts_d = nc.dram_tensor("wts", [NP, KK * 4 * NJB], BF16, kind="ExternalInput")
    out_d = nc.dram_tensor("out", [NP, HW], BF16, kind="ExternalOutput")
    tbl_d = nc.dram_tensor("tbl", [KK, TROWS, TCOLS], BF16, kind="Internal")

    TBL_KK = TROWS * TCOLS

    with tile.TileContext(nc) as tc, ExitStack() as ctx:
        cpool = ctx.enter_context(tc.tile_pool(name="const", bufs=1))
        tcst_pool = ctx.enter_context(tc.tile_pool(name="tcst", bufs=8))
        gpool = ctx.enter_context(tc.tile_pool(name="gat", bufs=3))
        ppool = ctx.enter_context(tc.tile_pool(name="prod", bufs=2))
        accpool = ctx.enter_context(tc.tile_pool(name="acc", bufs=1))
        pspool = ctx.enter_context(tc.tile_pool(name="ps", bufs=2, space="PSUM"))

        x_sb = cpool.tile([NP, XPAD], BF16)
        wt_sb = cpool.tile([NP, KK * F], BF16)
        idx_sb = cpool.tile([NP, KK * 256], I16)
        wts_sb = cpool.tile([NP, KK * 4 * NJB], BF16)
        acc_sb = accpool.tile([NP, NJB, F], BF16)

        # Load the gather ucode library up front so the first dma_gather
        # doesn't pay the ~6us IRAM load mid-pipeline.
        nc.gpsimd.load_library(library_config.mlp)

        nc.sync.dma_start(idx_sb[:], idx_d[:])
        nc.sync.dma_start(x_sb[:], x_d[:])
        nc.sync.dma_start(wt_sb[:], wt_d[:])
        nc.sync.dma_start(wts_sb[:], wts_d[:])

        # PE warmup: ~4.5us of dummy matmuls during the input DMAs flips the
        # HAM clock gate to 2.4 GHz before the kk0 table build starts.
        warm = cpool.tile([NP, NP], BF16)
        nc.vector.memset(warm[:], 0)
        wps = pspool.tile([NP, 4, 512], F32, tag="ps")
        for _ in range(42):
            nc.tensor.matmul(wps[:, 0, 0:NP], warm[:], warm[:], start=True, stop=True)

        # ---- Stage A: build TC tables, kk0 first.
        # per (group, tt, slot): stationary = shifted x tile; one matmul over
        # the group's kk's (N = 128*len(group)) -> psum -> evict (cast bf16,
        # 3:1 ACT:DVE) into tcst staging; one DMA per (group, tt) writes the
        # group's table rows.
        # kk0: batch 4 row-tiles per psum tile -> one evict + one DMA per 4
        # tiles. The serial evict chain gates the first gather's table, so
        # fewer, wider evictions shorten the pipeline head.
        for stt in range(9):
            tts = list(range(stt * 4, min(stt * 4 + 4, NTT)))
            ntt = len(tts)
            tcst = tcst_pool.tile([NP, 4, 4, F], BF16, tag="tcst0")
            ps = pspool.tile([NP, 4, 512], F32, tag="ps")
            for s in range(4):
                for t4, tt in enumerate(tts):
                    xoff = XPAD_LO + tt * NP + SLOT_SHIFT[s]
                    nc.tensor.matmul(
                        ps[:, s, t4 * F : (t4 + 1) * F],
                        x_sb[:, xoff : xoff + NP],
                        wt_sb[:, 0:F],
                        start=True,
                        stop=True,
                    )
            srcall = ps[:, :, 0 : ntt * F].rearrange("p s (t f) -> p s t f", t=ntt)
            nc.scalar.copy(
                tcst[:, 0:ntt, :, :].rearrange("p t s f -> p s t f"), srcall
            )
            dma_dst = bass.AP(
                tbl_d,
                stt * 4 * NP * TCOLS,
                [[TCOLS, NP], [NP * TCOLS, ntt], [1, TCOLS]],
            )
            nc.sync.dma_start(dma_dst, tcst[:, 0:ntt, :, :])

        for grp in KK_GROUPS[1:]:
            ng = len(grp)
            k0 = grp[0]
            for tt in range(NTT):
                tcst = tcst_pool.tile([NP, 3, 4, F], BF16, tag="tcst")
                ps = pspool.tile([NP, 4, 512], F32, tag="ps")
                for s in range(4):
                    xoff = XPAD_LO + tt * NP + SLOT_SHIFT[s]
                    nc.tensor.matmul(
                        ps[:, s, 0 : ng * F],
                        x_sb[:, xoff : xoff + NP],
                        wt_sb[:, k0 * F : (k0 + ng) * F],
                        start=True,
                        stop=True,
                    )
                # evict psum [p, s, (kk, f)] -> tcst [p, kk, s, f].
                # ALL on ACT: the DVE queue is strict in-order, so any DVE
                # eviction here would stall the stage-B combine behind it.
                srcall = ps[:, :, 0 : ng * F].rearrange("p s (k f) -> p s k f", k=ng)
                nc.scalar.copy(
                    tcst[:, 0:ng, :, :].rearrange("p k s f -> p s k f"), srcall
                )
                dma_dst = bass.AP(
                    tbl_d,
                    k0 * TBL_KK + tt * NP * TCOLS,
                    [[TCOLS, NP], [TBL_KK, ng], [1, TCOLS]],
                )
                nc.sync.dma_start(dma_dst, tcst[:, 0:ng, :, :])

        # ---- Stage B: one gather per kk + wide weighted accumulate on DVE.
        # The last kk is gathered in 2048-idx halves so its combine overlaps
        # the second half's dispatch, shortening the post-gather tail.
        NH = NJB // 2  # blocks per half
        for kk in range(KK):
            wbase = kk * 4 * NJB
            g_t = gpool.tile([NP, NJB, TCOLS], BF16, tag="g_t")
            src = bass.AP(tbl_d, kk * TBL_KK, [[TCOLS, HW + 65], [1, TCOLS]])
            parts = 4 if kk == KK - 1 else 1
            nh = NJB // parts
            for hh in range(parts):
                nc.gpsimd.dma_gather(
                    out_ap=g_t[:, hh * nh : (hh + 1) * nh, :],
                    in_ap=src,
                    idxs_ap=idx_sb[
                        :, kk * 256 + hh * (256 // parts) : kk * 256 + (hh + 1) * (256 // parts)
                    ],
                    num_idxs=HW // parts,
                    num_idxs_reg=HW // parts,
                    elem_size=TCOLS,
                    single_packet=False,
                    queue_num=kk % 2,
                )
                # acc[p, i, f] += sum_c w_c[p, i] * g_t[p, i, c*F:(c+1)*F]
                for ci in range(4):
                    w_b = (
                        wts_sb[:, wbase + ci * NJB + hh * nh : wbase + ci * NJB + (hh + 1) * nh]
                        .unsqueeze(-1)
                        .broadcast_to((NP, nh, F))
                    )
                    g_c = g_t[:, hh * nh : (hh + 1) * nh, ci * F : (ci + 1) * F]
                    a_h = acc_sb[:, hh * nh : (hh + 1) * nh, :]
                    if kk == 0 and ci == 0:
                        nc.vector.tensor_tensor(a_h, g_c, w_b, mybir.AluOpType.mult)
                    else:
                        prod = ppool.tile([NP, nh, F], BF16, tag="prod")
                        nc.vector.tensor_tensor(prod[:], g_c, w_b, mybir.AluOpType.mult)
                        nc.vector.tensor_tensor(a_h, a_h, prod[:], mybir.AluOpType.add)
                if kk == KK - 1:
                    # this quarter of acc is final: stream it out now so the
                    # output DMA overlaps the remaining quarters' combine.
                    nc.sync.dma_start(
                        out_d[:, hh * nh * F : (hh + 1) * nh * F],
                        acc_sb[:, hh * nh : (hh + 1) * nh, :],
                    )

    nc.compile()
    _split_overfull_waits(nc)
    return nc


_NC_CACHE = {}


def _get_nc():
    if "nc" not in _NC_CACHE:
        _NC_CACHE["nc"] = _build_nc()
    return _NC_CACHE["nc"]


def _prep_x(xb):
    """x [C,H,W] f32 -> padded bf16 [128, XPAD]."""
    xp = np.zeros((C, XPAD), ml_dtypes.bfloat16)
    xp[:, XPAD_LO : XPAD_LO + HW] = xb.reshape(C, HW).astype(ml_dtypes.bfloat16)
    return xp


def kernel(x, offset, mask, weight, **run_kwargs):
    x = np.asarray(x, np.float32)
    offset = np.asarray(offset, np.float32)
    mask = np.asarray(mask, np.float32)
    weight = np.asarray(weight, np.float32)

    wt = np.transpose(weight.reshape(F, C, KK), (1, 2, 0)).reshape(C, KK * F)
    wt = np.ascontiguousarray(wt).astype(ml_dtypes.bfloat16)

    in_maps = []
    for b in range(B):
        idx_dev, wts_dev = _prep_indices_weights(offset[b], mask[b])
        in_maps.append(
            {
                "x": _prep_x(x[b]),
                "wt": wt,
                "idx": idx_dev,
                "wts": wts_dev,
            }
        )

    nc = _get_nc()
    res = run_bass_kernel_spmd(nc, in_maps, core_ids=list(range(8)), **run_kwargs)
    # out_d[p, i*F + f] holds out[f, j = i*128 + p]: transpose on host.
    outs = []
    for b in range(B):
        arr = np.asarray(res.results[b]["out"]).astype(np.float32)
        arr = arr.reshape(NP, NJB, F)  # [p, i, f]
        outs.append(np.transpose(arr, (2, 1, 0)).reshape(F, H, W))
    out = np.stack(outs)
    if run_kwargs:
        kernel.last_results = res
    return out


# revision 19
# speedup vs baseline: 1.0618x; 1.0145x over previous
"""Deformable Conv2D (DCNv2-style) on 8 Trainium2 NeuronCores.

Strategy (data-parallel over batch, one sample per core):
  conv-first reformulation:  out[f,j] = sum_kk sum_corner w_corner[kk,j] * Y_kk[f, p_corner(kk,j)]
  where Y_kk = W[:,:,kk] @ x  (plain matmul over all spatial positions).

  Sampling uses a per-tap DRAM table TC[kk] whose row t packs the 4 bilinear
  corner pixel-vectors [Y(t-65) | Y(t-64) | Y(t-1) | Y(t)] (bf16, 1 KB), so a
  single dma_gather descriptor per (tap, output position) fetches all four
  corners.  Gather dispatch on GPSIMD (~7.5 ns/descriptor, 36864 descriptors,
  one ucode core-pair) is the hard critical path; everything else must hide
  under it and must not contend its SBUF port:
    - stage A is ordered kk0-first (then kk{1,2}, {3,4,5}, {6,7,8}) so the
      first gather's table is ready ~40us earlier;
    - the corner combine runs as WIDE DVE ops (one multiply per (kk, corner)
      over all 4096 output positions via a stride-0 broadcast weight view,
      plus wide adds into a bf16 SBUF accumulator) -- narrow per-block ops
      and PE-side accumulation were measured to slow the gather dispatch;
    - psum eviction runs entirely on ACT (the DVE queue is strict
      in-order, so DVE evictions would stall the combine behind them).

  The accumulator keeps j on partitions ([j%128, j//128, f] layout); the
  final [f, j] transpose is done on the host from the raw DMA-out, which
  removes the on-device transpose stage entirely.

Shapes (hardcoded per spec): x (8,128,64,64) f32, offset (8,18,64,64),
mask (8,9,64,64), weight (128,128,3,3), out (8,128,64,64) f32.
"""

import numpy as np
import ml_dtypes
from contextlib import ExitStack

import concourse.bass as bass
import concourse.bacc as bacc
import concourse.tile as tile
from concourse import mybir
from concourse import library_config
from concourse.bass_utils import run_bass_kernel_spmd

B, C, H, W = 8, 128, 64, 64
F = 128
KH = KW = 3
KK = KH * KW
HW = H * W  # 4096
NP = 128
NJB = HW // NP  # 32 j-blocks
NTT = 33  # table row tiles (t in [0, 4224)); gather uses rows [0, 4160]
TROWS = NTT * NP
TCOLS = 4 * F  # 512
# column-slot source shifts: TC[t] = [Y(t-65) | Y(t-64) | Y(t-1) | Y(t)]
SLOT_SHIFT = (-65, -64, -1, 0)
XPAD_LO = 65  # x padding so shifted tiles never index out of range
XPAD = XPAD_LO + TROWS + 64  # padded x columns
# stage-A kk grouping: kk0 alone first so gather kk0 can start early
KK_GROUPS = ((0,), (1, 2), (3, 4, 5), (6, 7, 8))  # [0] built separately

BF16 = mybir.dt.bfloat16
F32 = mybir.dt.float32
I16 = mybir.dt.int16


def _prep_indices_weights(offset, mask):
    """Per-sample host prep. offset [18,H,W], mask [9,H,W] ->
    idx int16 [128, KK*256], wts bf16 [128, KK*4*NJB]."""
    off = offset.reshape(KK, 2, H, W)
    dy, dx = off[:, 0], off[:, 1]
    ki, kj = np.meshgrid(np.arange(KH), np.arange(KW), indexing="ij")
    ki = ki.reshape(KK, 1, 1).astype(np.float32)
    kj = kj.reshape(KK, 1, 1).astype(np.float32)
    base_y = (np.arange(H, dtype=np.float32) - 1.0)[None, :, None] + ki
    base_x = (np.arange(W, dtype=np.float32) - 1.0)[None, None, :] + kj
    py = base_y + dy
    px = base_x + dx
    y0 = np.floor(py)
    x0 = np.floor(px)
    ly = (py - y0).astype(np.float32)
    lx = (px - x0).astype(np.float32)
    hy = 1.0 - ly
    hx = 1.0 - lx
    y0i = y0.astype(np.int64)
    x0i = x0.astype(np.int64)

    vy0 = (y0i >= 0) & (y0i < H)
    vy1 = (y0i + 1 >= 0) & (y0i + 1 < H)
    vx0 = (x0i >= 0) & (x0i < W)
    vx1 = (x0i + 1 >= 0) & (x0i + 1 < W)

    m = mask.reshape(KK, H, W)
    w00 = (hy * hx * m * (vy0 & vx0)).reshape(KK, HW).astype(np.float32)
    w01 = (hy * lx * m * (vy0 & vx1)).reshape(KK, HW).astype(np.float32)
    w10 = (ly * hx * m * (vy1 & vx0)).reshape(KK, HW).astype(np.float32)
    w11 = (ly * lx * m * (vy1 & vx1)).reshape(KK, HW).astype(np.float32)

    flat = np.clip(y0i * W + x0i + 65, 0, HW + 64).reshape(KK, HW)

    # idx: per kk, 4096 ordinals j wrapped o -> [o%16, o//16], replicated to
    # 128 partitions (dma_gather consumes idxs from each 16-partition group).
    idx_dev = np.empty((128, KK * 256), np.int16)
    for kk in range(KK):
        wrapped = flat[kk].astype(np.int16).reshape(256, 16).T  # [16, 256]
        idx_dev[:, kk * 256 : (kk + 1) * 256] = np.tile(wrapped, (8, 1))

    # wts: [128, (kk, corner, i)]; value[p] = w_c[kk, i*128+p]
    wts_dev = np.empty((128, KK * 4 * NJB), ml_dtypes.bfloat16)
    corners = (w00, w01, w10, w11)
    col = 0
    for kk in range(KK):
        for ci in range(4):
            wc = corners[ci][kk].reshape(NJB, 128)
            wts_dev[:, col : col + NJB] = wc.T
            col += NJB
    return idx_dev, wts_dev


def _split_overfull_waits(nc):
    """This walrus build accepts 1 sync-wait per instruction (2 for EVSEM).
    Move extras onto preceding same-engine NoOps."""
    for f in nc.m.functions:
        for bb in f.blocks:
            new_list = []
            for ins in bb.instructions:
                si = ins.sync_info
                waits = list(si.on_wait) if si and si.on_wait else []
                cap = 2 if isinstance(ins, mybir.InstEventSemaphore) else 1
                if len(waits) > cap:
                    extra, keep = waits[:-cap], waits[-cap:]
                    for k, w in enumerate(extra):
                        nop = mybir.InstNoOp(
                            name=f"{ins.name}_waitsplit{k}",
                            sync_info=mybir.SyncInfo(on_wait=[w], on_update=[]),
                            bass_nofuse=True,
                            engine=ins.engine,
                        )
                        new_list.append(nop)
                        nc.register_instruction(nop, overwrite=True)
                    si.on_wait = keep
                new_list.append(ins)
            bb.instructions[:] = new_list


def _build_nc():
    nc = bacc.Bacc(None, target_bir_lowering=False, debug=False, num_swdge_queues=2)
    x_d = nc.dram_tensor("x", [NP, XPAD], BF16, kind="ExternalInput")
    wt_d = nc.dram_tensor("wt", [NP, KK * F], BF16, kind="ExternalInput")
    idx_d = nc.dram_tensor("idx", [NP, KK * 256], I16, kind="ExternalInput")
    wts_d = nc.dram_tensor("wts", [NP, KK * 4 * NJB], BF16, kind="ExternalInput")
    out_d = nc.dram_tensor("out", [NP, HW], BF16, kind="ExternalOutput")
    tbl_d = nc.dram_tensor("tbl", [KK, TROWS, TCOLS], BF16, kind="Internal")

    TBL_KK = TROWS * TCOLS

    with tile.TileContext(nc) as tc, ExitStack() as ctx:
        cpool = ctx.enter_context(tc.tile_pool(name="const", bufs=1))
        tcst_pool = ctx.enter_context(tc.tile_pool(name="tcst", bufs=8))
        gpool = ctx.enter_context(tc.tile_pool(name="gat", bufs=3))
        ppool = ctx.enter_context(tc.tile_pool(name="prod", bufs=2))
        accpool = ctx.enter_context(tc.tile_pool(name="acc", bufs=1))
        pspool = ctx.enter_context(tc.tile_pool(name="ps", bufs=2, space="PSUM"))

        x_sb = cpool.tile([NP, XPAD], BF16)
        wt_sb = cpool.tile([NP, KK * F], BF16)
        idx_sb = cpool.tile([NP, KK * 256], I16)
        wts_sb = cpool.tile([NP, KK * 4 * NJB], BF16)
        acc_sb = accpool.tile([NP, NJB, F], BF16)

        # Load the gather ucode library up front so the first dma_gather
        # doesn't pay the ~6us IRAM load mid-pipeline.
        nc.gpsimd.load_library(library_config.mlp)

        nc.sync.dma_start(idx_sb[:], idx_d[:])
        nc.sync.dma_start(x_sb[:], x_d[:])
        nc.sync.dma_start(wt_sb[:], wt_d[:])
        nc.sync.dma_start(wts_sb[:], wts_d[:])

        # PE warmup: ~4.5us of dummy matmuls during the input DMAs flips the
        # HAM clock gate to 2.4 GHz before the kk0 table build starts.
        warm = cpool.tile([NP, NP], BF16)
        nc.vector.memset(warm[:], 0)
        wps = pspool.tile([NP, 4, 512], F32, tag="ps")
        for _ in range(42):
            nc.tensor.matmul(wps[:, 0, 0:NP], warm[:], warm[:], start=True, stop=True)

        # ---- Stage A: build TC tables, kk0 first.
        # per (group, tt, slot): stationary = shifted x tile; one matmul over
        # the group's kk's (N = 128*len(group)) -> psum -> evict (cast bf16,
        # 3:1 ACT:DVE) into tcst staging; one DMA per (group, tt) writes the
        # group's table rows.
        # kk0: batch 4 row-tiles per psum tile -> one evict + one DMA per 4
        # tiles. The serial evict chain gates the first gather's table, so
        # fewer, wider evictions shorten the pipeline head.
        for stt in range(9):
            tts = list(range(stt * 4, min(stt * 4 + 4, NTT)))
            ntt = len(tts)
            tcst = tcst_pool.tile([NP, 4, 4, F], BF16, tag="tcst0")
            ps = pspool.tile([NP, 4, 512], F32, tag="ps")
            for s in range(4):
                for t4, tt in enumerate(tts):
                    xoff = XPAD_LO + tt * NP + SLOT_SHIFT[s]
                    nc.tensor.matmul(
                        ps[:, s, t4 * F : (t4 + 1) * F],
                        x_sb[:, xoff : xoff + NP],
                        wt_sb[:, 0:F],
                        start=True,
                        stop=True,
                    )
            srcall = ps[:, :, 0 : ntt * F].rearrange("p s (t f) -> p s t f", t=ntt)
            nc.scalar.copy(
                tcst[:, 0:ntt, :, :].rearrange("p t s f -> p s t f"), srcall
            )
            dma_dst = bass.AP(
                tbl_d,
                stt * 4 * NP * TCOLS,
                [[TCOLS, NP], [NP * TCOLS, ntt], [1, TCOLS]],
            )
            nc.sync.dma_start(dma_dst, tcst[:, 0:ntt, :, :])

        for grp in KK_GROUPS[1:]:
            ng = len(grp)
            k0 = grp[0]
            for tt in range(NTT):
                tcst = tcst_pool.tile([NP, 3, 4, F], BF16, tag="tcst")
                ps = pspool.tile([NP, 4, 512], F32, tag="ps")
                for s in range(4):
                    xoff = XPAD_LO + tt * NP + SLOT_SHIFT[s]
                    nc.tensor.matmul(
                        ps[:, s, 0 : ng * F],
                        x_sb[:, xoff : xoff + NP],
                        wt_sb[:, k0 * F : (k0 + ng) * F],
                        start=True,
                        stop=True,
                    )
                # evict psum [p, s, (kk, f)] -> tcst [p, kk, s, f].
                # ALL on ACT: the DVE queue is strict in-order, so any DVE
                # eviction here would stall the stage-B combine behind it.
                srcall = ps[:, :, 0 : ng * F].rearrange("p s (k f) -> p s k f", k=ng)
                nc.scalar.copy(
                    tcst[:, 0:ng, :, :].rearrange("p k s f -> p s k f"), srcall
                )
                dma_dst = bass.AP(
                    tbl_d,
                    k0 * TBL_KK + tt * NP * TCOLS,
                    [[TCOLS, NP], [TBL_KK, ng], [1, TCOLS]],
                )
                nc.sync.dma_start(dma_dst, tcst[:, 0:ng, :, :])

        # ---- Stage B: one gather per kk + wide weighted accumulate on DVE.
        # The last kk is gathered in 2048-idx halves so its combine overlaps
        # the second half's dispatch, shortening the post-gather tail.
        NH = NJB // 2  # blocks per half
        for kk in range(KK):
            wbase = kk * 4 * NJB
            g_t = gpool.tile([NP, NJB, TCOLS], BF16, tag="g_t")
            src = bass.AP(tbl_d, kk * TBL_KK, [[TCOLS, HW + 65], [1, TCOLS]])
            parts = 4 if kk == KK - 1 else 1
            nh = NJB // parts
            for hh in range(parts):
                nc.gpsimd.dma_gather(
                    out_ap=g_t[:, hh * nh : (hh + 1) * nh, :],
                    in_ap=src,
                    idxs_ap=idx_sb[
                        :, kk * 256 + hh * (256 // parts) : kk * 256 + (hh + 1) * (256 // parts)
                    ],
                    num_idxs=HW // parts,
                    num_idxs_reg=HW // parts,
                    elem_size=TCOLS,
                    single_packet=False,
                    queue_num=kk % 2,
                )
                # acc[p, i, f] += sum_c w_c[p, i] * g_t[p, i, c*F:(c+1)*F]
                for ci in range(4):
                    w_b = (
                        wts_sb[:, wbase + ci * NJB + hh * nh : wbase + ci * NJB + (hh + 1) * nh]
                        .unsqueeze(-1)
                        .broadcast_to((NP, nh, F))
                    )
                    g_c = g_t[:, hh * nh : (hh + 1) * nh, ci * F : (ci + 1) * F]
                    a_h = acc_sb[:, hh * nh : (hh + 1) * nh, :]
                    if kk == 0 and ci == 0:
                        nc.vector.tensor_tensor(a_h, g_c, w_b, mybir.AluOpType.mult)
                    else:
                        prod = ppool.tile([NP, nh, F], BF16, tag="prod")
                        nc.vector.tensor_tensor(prod[:], g_c, w_b, mybir.AluOpType.mult)
                        nc.vector.tensor_tensor(a_h, a_h, prod[:], mybir.AluOpType.add)
                if kk == KK - 1:
                    # this quarter of acc is final: stream it out now so the
                    # output DMA overlaps the remaining quarters' combine.
                    nc.sync.dma_start(
                        out_d[:, hh * nh * F : (hh + 1) * nh * F],
                        acc_sb[:, hh * nh : (hh + 1) * nh, :],
                    )

    nc.compile()
    _split_overfull_waits(nc)
    return nc


_NC_CACHE = {}


def _get_nc():
    if "nc" not in _NC_CACHE:
        _NC_CACHE["nc"] = _build_nc()
    return _NC_CACHE["nc"]


def _prep_x(xb):
    """x [C,H,W] f32 -> padded bf16 [128, XPAD]."""
    xp = np.zeros((C, XPAD), ml_dtypes.bfloat16)
    xp[:, XPAD_LO : XPAD_LO + HW] = xb.reshape(C, HW).astype(ml_dtypes.bfloat16)
    return xp


def kernel(x, offset, mask, weight, **run_kwargs):
    x = np.asarray(x, np.float32)
    offset = np.asarray(offset, np.float32)
    mask = np.asarray(mask, np.float32)
    weight = np.asarray(weight, np.float32)

    wt = np.transpose(weight.reshape(F, C, KK), (1, 2, 0)).reshape(C, KK * F)
    wt = np.ascontiguousarray(wt).astype(ml_dtypes.bfloat16)

    in_maps = []
    for b in range(B):
        idx_dev, wts_dev = _prep_indices_weights(offset[b], mask[b])
        in_maps.append(
            {
                "x": _prep_x(x[b]),
                "wt": wt,
                "idx": idx_dev,
                "wts": wts_dev,
            }
        )

    nc = _get_nc()
    res = run_bass_kernel_spmd(nc, in_maps, core_ids=list(range(8)), **run_kwargs)
    # out_d[p, i*F + f] holds out[f, j = i*128 + p]: transpose on host.
    outs = []
    for b in range(B):
        arr = np.asarray(res.results[b]["out"]).astype(np.float32)
        arr = arr.reshape(NP, NJB, F)  # [p, i, f]
        outs.append(np.transpose(arr, (2, 1, 0)).reshape(F, H, W))
    out = np.stack(outs)
    if run_kwargs:
        kernel.last_results = res
    return out
